# revision 18
# baseline (speedup 1.0000x reference)
"""Causal self-attention (B=4, T=2048, C=1024, H=16, D=64, RoPE) on 8 trn2 cores.

Sharding: data-parallel over batch (4) x tensor-parallel over head-halves (2).
core = 2*b + hh handles batch b, heads [hh*8, hh*8+8).

v2 design (vs v1): QC=512 query chunks, per-head scores/PV matmuls (one PSUM
bank per output), diagonal key chunks narrowed to the causally-valid query
window (saves ~20us PE + ~20us ACT), exp-only on ACT, PSUM->SBUF copies on
the Pool (gpsimd) engine, softmax denominators via direct reciprocal from the
PSUM ones-row, broadcast via one K=33 matmul per (pair, round), and the
projection matmuls of round r+1 interleaved as "fillers" into round r's
attention stream so the in-order PE never stalls on the ACT exp pipeline.

Per-core structure (all matmuls bf16, fp32 PSUM):
  - QT/KT projection transposed [c_out, t] + RoPE (DVE), V natural with a
    ones column per head (65 cols/head) for free softmax row sums.
  - attention rounds r=0..3 over 512-query chunks; per (pair, j): scores^T
    per head into st [128k, 2, 512], exp (ACT), staircase mask multiply on
    the 4 diagonal chunks (DVE), PV accumulate per head into pv [65, 2, 512].
  - drain: reciprocal of ones-row -> sinv rows {0,32}, yt copies (Pool),
    broadcast matmul (lhsT [33,128]) -> yt normalize (DVE).
  - out projection row-parallel -> partial [T, C] fp32; host sums pairs.
"""

import os

import numpy as np
import ml_dtypes

import concourse.bass as bass
import concourse.mybir as mybir
import concourse.tile as tile
from concourse.bass_utils import run_bass_kernel_spmd

BF16 = mybir.dt.bfloat16
F32 = mybir.dt.float32
NP_BF16 = ml_dtypes.bfloat16

B, T, C = 4, 2048, 1024
H, D = 16, 64
HPC = 8          # heads per core
CPC = HPC * D    # 512 features per core
N_CORES = 8
QC = 512         # query chunk per attention round
KC = 128         # key chunk
NR = T // QC     # 4 rounds
ROPE_BASE = 10000.0

LAST_EXEC_NS = None
LAST_RESULTS = None


def _split_sync_waits(nc):
    """This walrus build accepts at most one sync wait per instruction; hoist
    extra waits onto same-engine NOPs inserted immediately before."""
    ctr = 0
    for bb in nc.main_func.blocks:
        insts = bb.instructions
        new = []
        changed = False
        for inst in insts:
            si = inst.sync_info
            waits = list(si.on_wait or []) if si is not None else []
            if len(waits) > 1:
                changed = True
                for w in waits[:-1]:
                    ctr += 1
                    nop = mybir.InstNoOp(
                        name=f"waitsplit_nop_{ctr}", ins=[], outs=[],
                        engine=inst.engine,
                    )
                    nop.sync_info = mybir.SyncInfo(on_wait=[w], on_update=[])
                    new.append(nop)
                inst.sync_info = mybir.SyncInfo(
                    on_wait=[waits[-1]], on_update=list(si.on_update or [])
                )
            new.append(inst)
        if changed:
            insts[:] = new


def _build_nc(split_waits=True):
    nc = bass.Bass()

    xT = nc.dram_tensor("xT", [C, T], BF16, kind="ExternalInput")
    wq = nc.dram_tensor("wq", [C, CPC], BF16, kind="ExternalInput")
    wk = nc.dram_tensor("wk", [C, CPC], BF16, kind="ExternalInput")
    wv = nc.dram_tensor("wv", [C, CPC], BF16, kind="ExternalInput")
    wc = nc.dram_tensor("wc", [CPC, C], BF16, kind="ExternalInput")
    cos2 = nc.dram_tensor("cos2", [128, T], BF16, kind="ExternalInput")
    ssin = nc.dram_tensor("ssin", [128, T], BF16, kind="ExternalInput")
    # staircase mask mk[i, h, w] = 1 if w >= i else 0 (same for both heads)
    msk = nc.dram_tensor("msk", [128, 2 * QC], BF16, kind="ExternalInput")
    e2m = nc.dram_tensor("e2m", [33, 128], BF16, kind="ExternalInput")
    out = nc.dram_tensor("out", [T, C], F32, kind="ExternalOutput")

    KB = C // 128          # 8 k-blocks over c_in
    NT = CPC // 128        # 4 head-pair tiles
    TT16 = T // 128        # 16 t tiles for V

    with tile.TileContext(nc) as tc:
        with (
            tc.tile_pool(name="singles", bufs=1) as singles,
            tc.tile_pool(name="xw", bufs=1) as xw,
            tc.tile_pool(name="big", bufs=1) as big,
        ):
            # ---- persistent tiles ----
            cos_sb = singles.tile([128, T], BF16)
            ssin_sb = singles.tile([128, T], BF16)
            msk_sb = singles.tile([128, 2 * QC], BF16)
            e2m_sb = singles.tile([33, 128], BF16)
            xT_all = xw.tile([128, KB, T], BF16, name="xTa")
            wq_all = xw.tile([128, KB, CPC], BF16, name="wqa")
            wk_all = xw.tile([128, KB, CPC], BF16, name="wka")
            wv_all = xw.tile([128, KB, CPC], BF16, name="wva")
            wc_all = xw.tile([128, NT, C], BF16, name="wca")
            xT_sb = [xT_all[:, kb] for kb in range(KB)]
            wq_sb = [wq_all[:, kb] for kb in range(KB)]
            wk_sb = [wk_all[:, kb] for kb in range(KB)]
            wv_sb = [wv_all[:, kb] for kb in range(KB)]
            wc_sb = [wc_all[:, cb] for cb in range(NT)]
            qt_sb = [big.tile([128, T], BF16, name=f"qt{i}") for i in range(NT)]
            kt_sb = [big.tile([128, T], BF16, name=f"kt{i}") for i in range(NT)]
            yt_sb = [big.tile([128, T], BF16, name=f"yt{i}") for i in range(NT)]
            vp_sb = [big.tile([128, HPC * 65], BF16, name=f"vp{tt}") for tt in range(TT16)]
            qbd_sb = [big.tile([128, 2, QC], BF16, name=f"qbd{p}") for p in range(NT)]
            sinv_sb = [big.tile([33, QC], BF16, name=f"sinv{p}") for p in range(NT)]

            mskv = msk_sb.rearrange("p (h q) -> p h q", h=2)

            # ---- input DMAs: interleave so the first Q-proj group's deps
            # land first; sync (SP) and scalar (ACT) are the two HWDGE queues
            # ACT's queue stays empty (its SEQ must be free for PSUM copies);
            # sync = HWDGE in need-order, gpsimd = SWDGE for the rest.
            xTr = xT.rearrange("(k p) t -> p k t", k=KB)
            nc.gpsimd.dma_start(out=cos_sb[:, 0:T // 2], in_=cos2[:, 0:T // 2])
            nc.gpsimd.dma_start(out=ssin_sb[:, 0:T // 2], in_=ssin[:, 0:T // 2])
            for kb in range(KB):
                rows = slice(kb * 128, (kb + 1) * 128)
                nc.sync.dma_start(out=xT_all[:, kb, 0:T // 2], in_=xT[rows, 0:T // 2])
                nc.sync.dma_start(out=wq_all[:, kb], in_=wq[rows, :])
                nc.gpsimd.dma_start(out=wv_all[:, kb], in_=wv[rows, :])
            for kb in range(KB):
                nc.sync.dma_start(out=wk_all[:, kb], in_=wk[kb * 128:(kb + 1) * 128, :])
            nc.sync.dma_start(out=msk_sb, in_=msk[:, :])
            nc.gpsimd.dma_start(out=e2m_sb, in_=e2m[:, :])
            for kb in range(KB):
                nc.gpsimd.dma_start(out=xT_all[:, kb, T // 2:],
                                    in_=xT[kb * 128:(kb + 1) * 128, T // 2:])
            nc.gpsimd.dma_start(out=cos_sb[:, T // 2:], in_=cos2[:, T // 2:])
            nc.gpsimd.dma_start(out=ssin_sb[:, T // 2:], in_=ssin[:, T // 2:])
            for cb in range(NT):
                nc.gpsimd.dma_start(out=wc_all[:, cb], in_=wc[cb * 128:(cb + 1) * 128, :])

            with (
                tc.tile_pool(name="pj_psum", bufs=2, space="PSUM") as pj_psum,
                tc.tile_pool(name="st_psum", bufs=2, space="PSUM") as st_psum,
                tc.tile_pool(name="pv_psum", bufs=1, space="PSUM") as pv_psum,
                tc.tile_pool(name="pr_pool", bufs=4) as pr_pool,
                tc.tile_pool(name="tmp", bufs=4) as tmp,
                tc.tile_pool(name="stg", bufs=5) as stg,
            ):
                # ---------- emission helpers ----------
                def proj_qk_group_thunks(w_sb, dst, i, r):
                    """Thunks: 8 matmuls + rope finish for one [128,512] tile."""
                    ts = slice(r * QC, (r + 1) * QC)
                    state = {}

                    def mk_mm(kb):
                        def f():
                            if kb == 0:
                                state["ps"] = pj_psum.tile([128, QC], F32, name="pj")
                            nc.tensor.matmul(
                                state["ps"],
                                lhsT=w_sb[kb][:, i * 128:(i + 1) * 128],
                                rhs=xT_sb[kb][:, ts],
                                start=(kb == 0),
                                stop=(kb == KB - 1),
                            )
                        return f

                    def fin():
                        ps = state["ps"]
                        raw = tmp.tile([128, QC], BF16, name="raw")
                        nc.scalar.copy(out=raw, in_=ps)
                        t1 = tmp.tile([128, QC], BF16, name="t1")
                        nc.vector.tensor_mul(t1, raw, cos_sb[:, ts])
                        rot = tmp.tile([128, QC], BF16, name="rot")
                        for rb in (0, 64):
                            nc.vector.tensor_scalar_mul(
                                rot[rb:rb + 32, :], raw[rb + 32:rb + 64, :], -1.0
                            )
                            nc.vector.tensor_copy(
                                out=rot[rb + 32:rb + 64, :], in_=raw[rb:rb + 32, :]
                            )
                        t2 = tmp.tile([128, QC], BF16, name="t2")
                        nc.vector.tensor_mul(t2, rot, ssin_sb[:, ts])
                        nc.gpsimd.tensor_add(dst[i][:, ts], t1, t2)
                    return [(213, 0, mk_mm(kb)) for kb in range(KB)] + [(0, 612, fin)]

                def proj_v_group_thunks(tt):
                    state = {}

                    def mk_mm(kb):
                        def f():
                            if kb == 0:
                                state["ps"] = pj_psum.tile([128, QC], F32, name="pj")
                            nc.tensor.matmul(
                                state["ps"],
                                lhsT=xT_sb[kb][:, tt * 128:(tt + 1) * 128],
                                rhs=wv_sb[kb][:, :],
                                start=(kb == 0),
                                stop=(kb == KB - 1),
                            )
                        return f

                    def fin():
                        ps = state["ps"]
                        vdst = vp_sb[tt].rearrange("p (h e) -> p h e", e=65)
                        nc.scalar.copy(
                            out=vdst[:, :, 0:64],
                            in_=ps.rearrange("p (h e) -> p h e", e=64),
                        )
                    return [(213, 0, mk_mm(kb)) for kb in range(KB)] + [(0, 612, fin)]

                def outproj_group_thunks(qt, co):
                    state = {}

                    def mk_mm(cb):
                        def f():
                            if cb == 0:
                                state["ps"] = pj_psum.tile([128, 512], F32, name="pj")
                            nc.tensor.matmul(
                                state["ps"],
                                lhsT=yt_sb[cb][:, qt * 128:(qt + 1) * 128],
                                rhs=wc_sb[cb][:, co * 512:(co + 1) * 512],
                                start=(cb == 0),
                                stop=(cb == NT - 1),
                            )
                        return f

                    def fin():
                        ps = state["ps"]
                        st_ = stg.tile([128, 512], F32, name="ost")
                        if (qt + co) % 2 == 0:
                            nc.scalar.copy(out=st_, in_=ps)
                            nc.sync.dma_start(
                                out=out[qt * 128:(qt + 1) * 128,
                                        co * 512:(co + 1) * 512],
                                in_=st_,
                            )
                        else:
                            nc.vector.tensor_copy(out=st_, in_=ps)
                            nc.gpsimd.dma_start(
                                out=out[qt * 128:(qt + 1) * 128,
                                        co * 512:(co + 1) * 512],
                                in_=st_,
                            )
                    act_fin = 612 if (qt + co) % 2 == 0 else 0
                    return [(213, 0, mk_mm(cb)) for cb in range(NT)] + [(0, act_fin, fin)]

                def proj_round_thunks(r):
                    th = []
                    for w_sb, dst in ((wq_sb, qt_sb), (wk_sb, kt_sb)):
                        for i in range(NT):
                            th.extend(proj_qk_group_thunks(w_sb, dst, i, r))
                    for tt in range(4 * r, 4 * r + 4):
                        th.extend(proj_v_group_thunks(tt))
                    return th

                def outproj_round_thunks(r):
                    th = []
                    for qt in range(4 * r, 4 * r + 4):
                        for co in range(2):
                            th.extend(outproj_group_thunks(qt, co))
                    return th

                # ---------- bootstrap: projections for round 0 ----------
                for _, _, f in proj_round_thunks(0):
                    f()

                # one-time zero/ones inits, needed just before attention r0
                for p in range(NT):
                    qbv = qbd_sb[p]
                    nc.gpsimd.memset(qbv[0:64, 1, :], 0.0)
                    nc.gpsimd.memset(qbv[64:128, 0, :], 0.0)
                    nc.gpsimd.memset(sinv_sb[p], 0.0)
                for tt in range(TT16):
                    vdst = vp_sb[tt].rearrange("p (h e) -> p h e", e=65)
                    nc.gpsimd.memset(vdst[:, :, 64:65], 1.0)

                # ---------- attention rounds with fillers ----------
                for r in range(NR):
                    fillers = []
                    if r < NR - 1:
                        fillers.extend(proj_round_thunks(r + 1))
                    else:
                        # round 3 has the largest exp deficit and no proj
                        # work left; feed it all the deferred out-proj rounds
                        for oc in range(NR - 1):
                            fillers.extend(outproj_round_thunks(oc))
                    deficit = 0.0  # est ACT ns minus est PE ns
                    pending_norm = []

                    q0 = r * QC
                    ts = slice(q0, q0 + QC)
                    njc = 4 * r + 4
                    for pair in range(NT):
                        qbv = qbd_sb[pair]
                        nc.vector.tensor_copy(
                            out=qbv[0:64, 0, :], in_=qt_sb[pair][0:64, ts]
                        )
                        nc.vector.tensor_copy(
                            out=qbv[64:128, 1, :], in_=qt_sb[pair][64:128, ts]
                        )
                        pv = pv_psum.tile([65, 2, QC], F32, name="pv")

                        def offw(j):
                            dj = j - 4 * r
                            off = 128 * dj if dj > 0 else 0
                            return off, QC - off

                        def emit_scores(j, st):
                            off, W = offw(j)
                            for h2 in range(2):
                                nc.tensor.matmul(
                                    st[:, h2, off:],
                                    lhsT=kt_sb[pair][:, j * KC:(j + 1) * KC],
                                    rhs=qbv[:, h2, off:],
                                    start=True,
                                    stop=True,
                                )

                        sts = [st_psum.tile([128, 2, QC], F32, name="st")]
                        emit_scores(0, sts[0])
                        for j in range(njc):
                            off, W = offw(j)
                            st = sts[j]
                            pr = pr_pool.tile([128, 2, QC], BF16, name="pr")
                            nc.scalar.activation(
                                out=pr[:, :, off:], in_=st[:, :, off:],
                                func=mybir.ActivationFunctionType.Exp, scale=0.125,
                            )
                            if j >= 4 * r:
                                # only the 128-wide stair at the window's left
                                # edge can be invalid (w' >= i holds beyond it)
                                nc.vector.tensor_mul(
                                    pr[:, :, off:off + KC],
                                    pr[:, :, off:off + KC],
                                    mskv[:, :, 0:KC],
                                )
                            if j + 1 < njc:
                                stn = st_psum.tile([128, 2, QC], F32, name="st")
                                sts.append(stn)
                                emit_scores(j + 1, stn)
                            deficit += (2 * W) * 0.8333 + 185 - 2 * W * 0.4167
                            while deficit > 0 and fillers:
                                pe_c, act_c, f = fillers.pop(0)
                                f()
                                deficit -= pe_c
                                deficit += act_c
                            for h2 in range(2):
                                h = 2 * pair + h2
                                nc.tensor.matmul(
                                    pv[:, h2, off:],
                                    lhsT=vp_sb[j][:, h * 65:h * 65 + 65],
                                    rhs=pr[:, h2, off:],
                                    start=(j == 0),
                                    stop=(j == njc - 1),
                                    skip_group_check=(j > 4 * r),
                                )
                        # drain pv: denominators + yt + normalize
                        with nc.allow_low_precision(reason="softmax denom bf16"):
                            nc.vector.reciprocal(
                                out=sinv_sb[pair][0:1, :], in_=pv[64:65, 0, :]
                            )
                            nc.vector.reciprocal(
                                out=sinv_sb[pair][32:33, :], in_=pv[64:65, 1, :]
                            )
                        nc.vector.tensor_copy(
                            out=yt_sb[pair][0:64, ts], in_=pv[0:64, 0, :]
                        )
                        nc.vector.tensor_copy(
                            out=yt_sb[pair][64:128, ts], in_=pv[0:64, 1, :]
                        )
                        def mk_norm(pair, ts):
                            def f():
                                bcp = pj_psum.tile([128, QC], F32, name="pj")
                                nc.tensor.matmul(
                                    bcp, lhsT=e2m_sb, rhs=sinv_sb[pair],
                                    start=True, stop=True,
                                )
                                bcs = tmp.tile([128, QC], BF16, name="bcs")
                                nc.vector.tensor_copy(out=bcs, in_=bcp)
                                nc.vector.tensor_mul(
                                    yt_sb[pair][:, ts], yt_sb[pair][:, ts], bcs
                                )
                            return f
                        pending_norm.append(mk_norm(pair, ts))
                        if len(pending_norm) > 1:
                            pending_norm.pop(0)()
                    # flush leftover fillers first (gives DVE time to retire
                    # the last pair's reciprocals), then the deferred norms
                    for _, _, f in fillers:
                        f()
                    for f in pending_norm:
                        f()
                    pending_norm.clear()
                # tail: out-proj of the last round
                for _, _, f in outproj_round_thunks(NR - 1):
                    f()
    if split_waits:
        _split_sync_waits(nc)
    return nc


_NC = None


def _host_tables():
    inv_freq = 1.0 / (ROPE_BASE ** (np.arange(0, D, 2, dtype=np.float32) / D))
    t = np.arange(T, dtype=np.float32)
    freqs = np.einsum("i,j->ij", t, inv_freq)          # [T, 32]
    emb = np.concatenate([freqs, freqs], axis=-1)      # [T, 64]
    cosT = np.cos(emb).T.astype(np.float32)            # [64, T]
    sinT = np.sin(emb).T.astype(np.float32)
    cos2 = np.concatenate([cosT, cosT], axis=0)        # [128, T]
    ssin = np.concatenate([sinT, sinT], axis=0)        # [128, T]

    # staircase causal mask on probs^T [128 keys, w]: valid iff w >= i
    i_ = np.arange(KC)[:, None]
    w_ = np.arange(QC)[None, :]
    mk1 = (w_ >= i_).astype(np.float32)                # [128, 512]
    msk = np.concatenate([mk1, mk1], axis=1)           # [128, 2*512]

    e2m = np.zeros((33, 128), dtype=np.float32)
    e2m[0, 0:64] = 1.0
    e2m[32, 64:128] = 1.0
    return cos2, ssin, msk, e2m


def kernel(x, Wq, Wkv, Wc):
    global _NC, LAST_EXEC_NS, LAST_RESULTS
    x = np.asarray(x, dtype=np.float32)
    Wq = np.asarray(Wq, dtype=np.float32)
    Wkv = np.asarray(Wkv, dtype=np.float32)
    Wc = np.asarray(Wc, dtype=np.float32)

    if _NC is None:
        _NC = _build_nc()

    cos2, ssin, msk, e2m = _host_tables()
    bf = lambda a: np.ascontiguousarray(a).astype(NP_BF16)

    in_maps = []
    for core in range(N_CORES):
        b, hh = core // 2, core % 2
        h0 = hh * HPC
        cols = slice(h0 * D, h0 * D + CPC)
        vcols = slice(C + h0 * D, C + h0 * D + CPC)
        in_maps.append(
            {
                "xT": bf(x[b].T),
                "wq": bf(Wq[:, cols]),
                "wk": bf(Wkv[:, cols]),
                "wv": bf(Wkv[:, vcols]),
                "wc": bf(Wc[cols.start:cols.stop, :]),
                "cos2": bf(cos2),
                "ssin": bf(ssin),
                "msk": bf(msk),
                "e2m": bf(e2m),
            }
        )

    trace = os.environ.get("BASS_PROF", "0") == "1"
    res = run_bass_kernel_spmd(_NC, in_maps, list(range(N_CORES)), trace=trace)
    LAST_EXEC_NS = res.exec_time_ns
    LAST_RESULTS = res
    y = np.empty((B, T, C), dtype=np.float32)
    for b in range(B):
        y[b] = res.results[2 * b]["out"] + res.results[2 * b + 1]["out"]
    return y
